# revision 48
# baseline (speedup 1.0000x reference)
"""Concordance CC (segment_reduce) Trainium2 Bass kernel — V20.

Problem: y_true, y_pred [256, 65536] f32, 0/1 validity mask [256, 65536] i32.
Per row: masked means/variances/covariance (ddof=1), ccc = 2*cov /
(var_t + var_p + 2*(mean_t - mean_p)); output = mean(ccc) (scalar f32).

Per-row stats are inner products over the valid range of columns of
W = [a-slots 0..63 | b-slots 0..63 | ones] with a = y_true*mask,
b = y_pred*mask: S2t=a.a  Stp=a.b  S1t=a.ones  S2p=b.b  S1p=b.ones —
read off an asymmetric Gram lhsT^T rhs with lhsT = the 128 data
columns and rhs = data + ones (129): out[128, 129] holds every needed
entry (diagonals, a.b diagonal, and the S1 column); cross-slot
entries are computed but ignored. O(B) scalar epilogue on host.

Masks are prefix-valid with lengths ~U[T/4, T] (mean 0.625*T), so
~37% of a dense stream would be zeros that still cost HBM traffic
and matmuls. kernel() sees the mask before compiling, so it SIZES
THE PROGRAM TO THE DATA: rows are cut into segments of S = 128*C
valid t-positions and bin-packed into the 8 cores x 2 phases x 64
Gram column-slot pairs, with C (chunks per core) chosen at runtime
as the smallest multiple of 32 whose slot capacity fits (seed-0
oracle input: C=96, i.e. 75% of the dense C=128). The two PHASES are
sequential chunk ranges accumulated into separate PSUM banks — a
column slot can therefore host a different row segment in each
phase, which is what makes the packing dense; raw moments are
additive, so the host just sums each row's slots. Phase 0's
writeback overlaps phase 1's matmuls; only phase 1's chain sits on
the tail.

The matmul cost on this part is CONTRACTION-bound (~77 ns ramped /
~127 ns cold per 256-deep DoubleRow chunk, independent of column
count up to the 128-col stationary limit), and the ~320 GB/s
per-core HBM share delivers a unit 2-3x slower than the PE eats it —
so C=96 cuts BOTH walls by 25%: 3.05 MiB and 96 matmuls per core.

Operand precision is FP8 E4M3 in DoubleRow perf mode (two
contraction sub-rows per partition -> one matmul contracts 256
t-positions). e4m3's ~3.6% rms quantization error lands at ~8.4e-3
relative on the final CCC for the seed-0 oracle input (measured in
simulation and on hardware; deterministic) — 2.4x inside the 2e-2
gate, for half the HBM traffic and half the PE instructions of bf16.

s3_lw_dual_fp8 ISA restrictions shape the layout: the two sub-rows
live in separate half-tile blocks per 16-chunk unit (outer AP step =
16*KP = 2080 B, 16B-aligned and even) and chunks are padded
129 -> 130 B so every chunk base stays 2B-aligned (violations
compile but kill the exec unit at runtime).

Schedule: full-unit pieces round-robin over 3 HWDGE rings (sync,
scalar, gpsimd) — fewest DIRECT2D generations per byte — with a
4+12 head split (the PE starts ~1.3 us sooner, which pays on
unramped-clock runs where the PE end is the critical path) and a
12+4 tail split so little PE work remains after the last byte. The
5-deep tile pool's recycle dependency paces the stream against the
PE while leaving one unit of slack (fully resident floods the
shared DMA engines; cross-ring gates measured neutral-to-worse —
descriptor service order is not controllable from the program).
Within a phase consecutive matmuls alternate between two full-bank
PSUM accumulators (~2 ns/MM cheaper than same-bank accumulate);
each of the 4 banks is written back on its own ring with no DVE add
on the critical path (host sums them). Keep the tail copies on DVE:
a nc.scalar.copy there lowers to an Activation instruction whose
first-use ACT table load costs ~1.4 us.

L = sum(mask) is a pure function of the mask, computed on host in
the same pass that marshals/quantizes it.
"""

import numpy as np

import concourse.bass as bass
import concourse.tile as tile
from concourse import mybir
from concourse.bass_utils import run_bass_kernel_spmd

# ---------------------------------------------------------------- constants
B, T = 256, 65536
NCORES = 8
R = 64                     # Gram column-slot pairs per core
CH = 16                    # DoubleRow chunks per unit (256 t each)
GCOLS = 2 * R + 1          # 129 Gram columns: [a slots | b slots | ones]
KP = GCOLS + 1             # chunk stride 130 B: keeps every chunk base even
                           # (s3_lw_dual_fp8: rhs base must be 2B-aligned)
UB = 2 * CH * KP           # unit free bytes: [2 sub-row blocks][CH][130]
KONE = 2 * R               # ones column
NPH = 2                    # sequential accumulation phases (slot reuse)

FP = mybir.dt.float8e4     # e4m3: Gram operand precision (DoubleRow-capable)
NPFP8 = mybir.dt.np(FP)    # numpy view (ml_dtypes.float8_e4m3)


def split_multi_waits(nc: bass.Bass) -> int:
    """This container's walrus build accepts at most ONE sync-wait per
    instruction, but Tile's sem assignment attaches all required waits to
    the consuming instruction. Hoist the excess onto same-engine NoOps
    inserted immediately before it (sequencers execute in order, so the
    waits are still satisfied before the instruction issues)."""
    n_split = 0
    for f in nc.m.functions:
        for bb in f.blocks:
            insts = bb.instructions
            out = []
            for inst in insts:
                si = inst.sync_info
                if si is not None and si.on_wait and len(si.on_wait) > 1:
                    waits = list(si.on_wait)
                    for w in waits[:-1]:
                        nop = mybir.InstNoOp(
                            name=f"I-wsplit-{nc.next_id()}", ins=[], outs=[]
                        )
                        nop.engine = inst.engine
                        nop.sync_info = mybir.SyncInfo(on_wait=[w], on_update=[])
                        out.append(nop)
                        n_split += 1
                    inst.sync_info = mybir.SyncInfo(
                        on_wait=[waits[-1]], on_update=list(si.on_update or [])
                    )
                out.append(inst)
            bb.instructions = out
    return n_split


def build_nc(nunits: int) -> bass.Bass:
    nc = bass.Bass()
    # host-marshaled Gram operand, staged per 16-chunk unit as two
    # contraction sub-row blocks (outer DoubleRow AP step = CH*KP =
    # 2080 B, the 16B-aligned even stride s3_lw_dual_fp8 demands):
    # w[u*128 + p, i*CH*KP + c*KP + k] = W_k(q = u*4096 + p*32 + c*2 + i)
    wpk = nc.dram_tensor("wpk", [nunits * 128, UB], FP,
                         kind="ExternalInput")
    # one partial Gram per PSUM accumulator; host sums them
    grams = [
        nc.dram_tensor(f"gram{i}", [2 * R, GCOLS], mybir.dt.float32,
                       kind="ExternalOutput")
        for i in range(2 * NPH)
    ]

    # full-unit pieces (fewest DIRECT2D generations per byte); the head
    # is split 4+12 so the PE starts ~1.3 us sooner (mid-clock runs are
    # PE-end-bound), the tail 12+4 so little PE work remains after the
    # last byte lands
    pieces = [(0, 0, 4), (0, 4, 12)]
    pieces += [(u, 0, CH) for u in range(1, nunits - 1)]
    pieces += [(nunits - 1, 0, 12), (nunits - 1, 12, 4)]
    total_mm = nunits * CH
    c2 = total_mm // 2     # phase boundary (nunits is even)

    with tile.TileContext(nc) as tc:
        with (
            tc.tile_pool(name="stage", bufs=5) as stage,
            tc.tile_pool(name="psum", bufs=1, space="PSUM") as psum,
            tc.tile_pool(name="outp", bufs=1) as outp,
        ):
            # 2 phases x 2 alternating full-bank PSUM accumulators
            pbank = [
                psum.tile([2 * R, 512], mybir.dt.float32, name=f"pbank{i}")
                for i in range(2 * NPH)
            ]
            ogs = [
                outp.tile([2 * R, GCOLS], mybir.dt.float32, name=f"og{i}")
                for i in range(2 * NPH)
            ]
            wrings = [nc.scalar, nc.gpsimd]

            nmm = 0
            rings = [nc.sync, nc.scalar, nc.gpsimd]
            sub = lambda ap: ap.rearrange("p (two ck) -> p two ck", two=2)

            for pi, (u, c0, cl) in enumerate(pieces):
                rows = slice(u * 128, (u + 1) * 128)
                # tiles are always full-size (uniform pool slots); tapered
                # pieces use only cl chunks of each sub-row block
                gt = stage.tile([128, UB], FP)
                if cl == CH:
                    rings[pi % 3].dma_start(out=gt[:, :], in_=wpk[rows, :])
                else:
                    csl = slice(c0 * KP, (c0 + cl) * KP)
                    rings[pi % 3].dma_start(
                        out=sub(gt[:, :])[:, :, csl],
                        in_=sub(wpk[rows, :])[:, :, csl],
                    )

                for ci in range(c0, c0 + cl):
                    phase = 0 if nmm < c2 else 1
                    bk = 2 * phase + (nmm % 2)
                    lhsT = sub(gt[:, :])[:, :, ci * KP : ci * KP + 2 * R]
                    rhs = sub(gt[:, :])[:, :, ci * KP : ci * KP + GCOLS]
                    nc.tensor.matmul(
                        pbank[bk][:, :GCOLS],
                        lhsT=lhsT,
                        rhs=rhs,
                        start=(nmm < 2 or c2 <= nmm < c2 + 2),
                        stop=(c2 - 2 <= nmm < c2 or nmm >= total_mm - 2),
                        perf_mode=mybir.MatmulPerfMode.DoubleRow,
                    )
                    nmm += 1
                    if nmm == c2:
                        # phase 0 done: its writeback chains (DVE copy +
                        # DMA gen) run under phase 1's matmuls, leaving
                        # only phase 1's chains on the tail
                        for i in range(2):
                            nc.vector.tensor_copy(
                                out=ogs[i][:, :], in_=pbank[i][:, :GCOLS]
                            )
                            wrings[i].dma_start(out=grams[i][:, :], in_=ogs[i][:, :])

            for i in range(2, 4):
                nc.vector.tensor_copy(out=ogs[i][:, :], in_=pbank[i][:, :GCOLS])
                wrings[i % 2].dma_start(out=grams[i][:, :], in_=ogs[i][:, :])
    split_multi_waits(nc)
    return nc


_NC_CACHE: dict = {}


def _get_nc(nunits: int):
    if nunits not in _NC_CACHE:
        _NC_CACHE[nunits] = build_nc(nunits)
    return _NC_CACHE[nunits]


def _choose_nunits(ell: np.ndarray) -> int:
    """Smallest per-core chunk count C (multiple of 32, so each of the 2
    phases is a whole number of 16-chunk units) whose 8*2*64 slot pairs of
    S = 128*C valid t-positions hold every row segment."""
    for c in range(32, 129, 32):
        if int(np.ceil(ell / (128.0 * c)).sum()) <= NCORES * NPH * R:
            return c // CH
    return 128 // CH


def _pack(y_true, y_pred, mask):
    """Segment rows into S-length pieces of their valid prefix, bin-pack
    into (core, phase, slot) in order, and marshal into the per-core
    DoubleRow unit layout. Returns (wpk [NCORES, nunits*128, UB], smap
    rows of (core, phase, slot, row), nunits)."""
    m = mask.astype(np.float32, copy=False)
    ell = mask.sum(axis=1)
    nunits = _choose_nunits(ell)
    pu = nunits // NPH            # units per phase
    S = pu * 4096                 # valid t-capacity per slot (= 128*C)

    a_full = (y_true * m).astype(NPFP8)
    b_full = (y_pred * m).astype(NPFP8)
    slots_a = np.zeros((NCORES, NPH, R, S), dtype=NPFP8)
    slots_b = np.zeros((NCORES, NPH, R, S), dtype=NPFP8)
    smap = []
    s = 0
    for r in range(B):
        t0, L = 0, int(ell[r])
        while t0 < L:
            ln = min(S, L - t0)
            core, phase, k = s // (NPH * R), (s % (NPH * R)) // R, s % R
            slots_a[core, phase, k, :ln] = a_full[r, t0 : t0 + ln]
            slots_b[core, phase, k, :ln] = b_full[r, t0 : t0 + ln]
            smap.append((core, phase, k, r))
            t0 += ln
            s += 1
    assert s <= NCORES * NPH * R

    # [core, phase, slot, pu, p, c, i] -> [core, phase, pu, p, i, c, slot]
    lay = lambda x: (
        x.reshape(NCORES, NPH, R, pu, 128, CH, 2).transpose(0, 1, 3, 4, 6, 5, 2)
    )
    a, b = lay(slots_a), lay(slots_b)
    w = np.zeros((NCORES, NPH, pu, 128, 2, CH, KP), dtype=NPFP8)
    w[..., 0:R] = a
    w[..., R : 2 * R] = b
    w[..., KONE] = np.float32(1.0)
    return w.reshape(NCORES, nunits * 128, UB), smap, nunits


def _ccc_from_outputs(results, smap, ell) -> np.ndarray:
    # per-(core, phase) Gram: sum of its two alternating PSUM banks
    gg = [
        [
            res["gram0"].astype(np.float64) + res["gram1"].astype(np.float64),
            res["gram2"].astype(np.float64) + res["gram3"].astype(np.float64),
        ]
        for res in results
    ]
    s2t = np.zeros(B)
    s2p = np.zeros(B)
    stp = np.zeros(B)
    s1t = np.zeros(B)
    s1p = np.zeros(B)
    for core, phase, k, r in smap:
        g = gg[core][phase]
        s2t[r] += g[k, k]
        s2p[r] += g[R + k, R + k]
        stp[r] += g[k, R + k]
        s1t[r] += g[k, KONE]
        s1p[r] += g[R + k, KONE]
    mean_t = s1t / ell
    mean_p = s1p / ell
    denom = ell - 1.0
    var_t = (s2t - s1t * s1t / ell) / denom
    var_p = (s2p - s1p * s1p / ell) / denom
    cov = (stp - s1t * s1p / ell) / denom
    ccc = 2.0 * cov / (var_t + var_p + (mean_t - mean_p) * 2.0)
    return np.float32(ccc.sum() / B)


def _in_maps(y_true, y_pred, mask):
    wp, smap, nunits = _pack(
        np.asarray(y_true), np.asarray(y_pred), np.asarray(mask)
    )
    return [{"wpk": wp[core]} for core in range(NCORES)], smap, nunits


def kernel(y_true, y_pred, mask) -> np.ndarray:
    mask = np.asarray(mask)
    # per-row valid length: a pure function of the mask, folded into the
    # same host pass that marshals/quantizes it
    ell = mask.sum(axis=1, dtype=np.int64).astype(np.float64)
    in_maps, smap, nunits = _in_maps(y_true, y_pred, mask)
    nc = _get_nc(nunits)
    res = run_bass_kernel_spmd(nc, in_maps, core_ids=list(range(NCORES)))
    return _ccc_from_outputs(res.results, smap, ell)
